# revision 5
# baseline (speedup 1.0000x reference)
"""Decoder-only transformer forward on 8 trn2 NeuronCores.

Sharding (SPMD, two small AllToAlls per layer, no AllGather):
  - residual stream token-sharded: core c owns flat tokens [256c, 256c+256)
  - QKV computed locally on own tokens with FULL Wq/Wk/Wv (bf16), then one
    AllToAll redistributes q,k,v head-sharded: core c gets heads (2c, 2c+1)
    for all 2048 tokens
  - attention head-sharded; ctx AllToAll'd back token-sharded
  - Wo / FFN token-sharded (full bf16 weights streamed per core)
  - LM head token-sharded: full Wout (bf16) streamed per core, logits out
    bf16 -- no final collective.

Perf structure:
  - LN gamma folded host-side into consumer weights (Wq/Wk/Wv, W1, Wout);
    LN beta folded into per-consumer bias terms, so LN on device is just
    (x - mean) * rstd (2 DVE ops/chunk); squares on the Act engine.
  - pad-key bias and per-block biases ride the Act engine's activation
    bias operand (per-partition) instead of separate DVE passes.
  - residual adds: PSUM + bias via Act Identity, then one DVE add.
  - A2A staging uses consolidated multi-dim DMAs issued from the Pool
    (gpsimd) queue; FFN weights stream from the Act queue; QKV/Wo/Wout
    weights from the SP queue -- three independent DMA paths.
  - matmul weights/activations bf16; PSUM fp32; residual fp32r.
"""

import math
import os

import numpy as np
import ml_dtypes

B, S, D, H, L, F, V = 2, 1024, 1024, 16, 6, 4096, 32000
NCORES = 8
T = B * S                 # 2048 flat tokens
TPC = T // NCORES         # 256 tokens per core
VPAD = 32768
DK = D // H               # 64
NEG = -1e9
EPS = 1e-5

_CACHE = {}


def _build():
    import concourse.mybir as mybir
    import concourse.tile as tile
    from concourse import bacc

    dtr = mybir.dt.float32r
    dtf = mybir.dt.float32
    dtb = mybir.dt.bfloat16

    nc = bacc.Bacc(
        "TRN2",
        target_bir_lowering=False,
        debug=False,
        enable_asserts=False,
        num_devices=NCORES,
    )
    RG = [list(range(NCORES))]

    # ---- I/O ----
    x0t_i = nc.dram_tensor("x0t", [D, TPC], dtr, kind="ExternalInput")
    wq_i = nc.dram_tensor("wq", [L, D, D], dtb, kind="ExternalInput")
    wk_i = nc.dram_tensor("wk", [L, D, D], dtb, kind="ExternalInput")
    wv_i = nc.dram_tensor("wv", [L, D, D], dtb, kind="ExternalInput")
    wo_i = nc.dram_tensor("wo", [L, D, D], dtb, kind="ExternalInput")
    w1_i = nc.dram_tensor("w1", [L, D, F], dtb, kind="ExternalInput")
    w2_i = nc.dram_tensor("w2", [L, F, D], dtb, kind="ExternalInput")
    wout_i = nc.dram_tensor("wout", [D, VPAD], dtb, kind="ExternalInput")
    # per-feature params in T layout ([128, n_chunks] per layer)
    bq_i = nc.dram_tensor("bq", [L, 128, 8], dtf, kind="ExternalInput")
    bk_i = nc.dram_tensor("bk", [L, 128, 8], dtf, kind="ExternalInput")
    vb_i = nc.dram_tensor("vb", [L, 1, 1024], dtb, kind="ExternalInput")
    bo_i = nc.dram_tensor("bo", [L, 128, 8], dtf, kind="ExternalInput")
    b1_i = nc.dram_tensor("b1", [L, 128, 32], dtf, kind="ExternalInput")
    b2_i = nc.dram_tensor("b2", [L, 128, 8], dtf, kind="ExternalInput")
    bout_i = nc.dram_tensor("bout", [128, 256], dtf, kind="ExternalInput")
    causal_i = nc.dram_tensor("causal", [128, 128], dtf, kind="ExternalInput")
    padb_i = nc.dram_tensor("padb", [2, 8, 128], dtf, kind="ExternalInput")
    # logits packed [128, 65536]: vocab v = 1024*vg + 128*q8 + p lives at
    # row p, col 2048*vg + 256*q8 + t  (4 KB DMA lines instead of 512 B)
    out_o = nc.dram_tensor("logits_t", [128, VPAD * TPC // 128], dtb,
                           kind="ExternalOutput")

    with tile.TileContext(nc) as tc:
        _body(
            nc, tc, mybir, dtr, dtf, dtb, RG,
            x0t_i, wq_i, wk_i, wv_i, wo_i, w1_i, w2_i, wout_i,
            bq_i, bk_i, vb_i, bo_i, b1_i, b2_i, bout_i,
            causal_i, padb_i, out_o,
        )
    nc.compile()
    return nc


def _body(nc, tc, mybir, dtr, dtf, dtb, RG,
          x0t_i, wq_i, wk_i, wv_i, wo_i, w1_i, w2_i, wout_i,
          bq_i, bk_i, vb_i, bo_i, b1_i, b2_i, bout_i,
          causal_i, padb_i, out_o):
    import contextlib
    AF = mybir.ActivationFunctionType
    OP = mybir.AluOpType
    ctx = contextlib.ExitStack()
    with ctx:
        const = ctx.enter_context(tc.tile_pool(name="const", bufs=1))
        lnp = ctx.enter_context(tc.tile_pool(name="lnp", bufs=1))
        resid = ctx.enter_context(tc.tile_pool(name="resid", bufs=1))
        act = ctx.enter_context(tc.tile_pool(name="act", bufs=1))
        act2 = ctx.enter_context(tc.tile_pool(name="act2", bufs=2))
        expp = ctx.enter_context(tc.tile_pool(name="expp", bufs=2))
        wqkv = ctx.enter_context(tc.tile_pool(name="wqkv", bufs=1))
        wff = ctx.enter_context(tc.tile_pool(name="wff", bufs=2))
        stat = ctx.enter_context(tc.tile_pool(name="stat", bufs=1))
        ps = ctx.enter_context(tc.tile_pool(name="ps", bufs=4, space="PSUM"))
        dram = ctx.enter_context(tc.tile_pool(name="dram", bufs=2, space="DRAM"))

        # ---- constants (staged through one f32 scratch tag) ----
        stage = const.tile([128, 128], dtf, tag="stage")
        nc.vector.memset(stage[:, 0:1], 1.0)
        ones_col = const.tile([128, 1], dtr, tag="ones_col")
        nc.scalar.copy(out=ones_col[:], in_=stage[:, 0:1])
        onesb_col = const.tile([128, 1], dtb, tag="onesb_col")
        nc.scalar.copy(out=onesb_col[:], in_=stage[:, 0:1])
        nc.vector.memset(stage[0:1, :], 1.0)
        ones_row = const.tile([1, 128], dtr, tag="ones_row")
        nc.scalar.copy(out=ones_row[:], in_=stage[0:1, :])
        onesb_row = const.tile([1, 128], dtb, tag="onesb_row")
        nc.scalar.copy(out=onesb_row[:], in_=stage[0:1, :])
        eps_t = const.tile([1, 1], dtf, tag="eps_t")
        nc.vector.memset(eps_t[:], 1e-5)

        causal = const.tile([128, 128], dtf, tag="causal")
        nc.gpsimd.dma_start(out=causal[:], in_=causal_i[:, :])
        padb = const.tile([128, 16], dtf, tag="padb")
        nc.gpsimd.dma_start(out=padb[:],
                            in_=padb_i.ap().rearrange("b k p -> p (b k)"))

        # ---- persistent residual: feature-major, chunk i = cols [256i, 256i+256) ----
        xT = resid.tile([128, 2048], dtr, tag="xT")
        nc.sync.dma_start(
            out=xT[:].rearrange("p (i t) -> p i t", i=8),
            in_=x0t_i.ap().rearrange("(i p) t -> p i t", p=128),
        )

        def cs(ap, i, w=256):
            return ap[:, i * w:(i + 1) * w]

        def layer_norm(src_tile):
            """src [128, 2048] f32r -> normalized (x-m)*rstd [128, 2048] bf16."""
            p_s = ps.tile([128, 1024], dtf, tag="bank")
            for i in range(8):
                sq = act2.tile([128, 256], dtr, tag="ln_sq")
                nc.scalar.activation(out=sq[:], in_=cs(src_tile[:], i),
                                     func=AF.Square)
                nc.tensor.matmul(p_s[0:1, 0:256], ones_col[:],
                                 cs(src_tile[:], i),
                                 start=(i == 0), stop=(i == 7))
                # start=True clears has_written for the whole bank, so the
                # sq group must NOT restart it: first write lands on cleared
                # bits (overwrite), later writes accumulate.
                nc.tensor.matmul(p_s[0:1, 256:512], ones_col[:], sq[:],
                                 start=False, stop=(i == 7))
            mean_r = stat.tile([1, 256], dtr, tag="mean_r")
            nc.scalar.activation(out=mean_r[:], in_=p_s[0:1, 0:256],
                                 func=AF.Copy, scale=1.0 / 1024.0)
            sc = stat.tile([1, 1024], dtf, tag="sc")  # ex2|msq|var|std
            nc.scalar.activation(out=sc[0:1, 0:256], in_=p_s[0:1, 256:512],
                                 func=AF.Copy, scale=1.0 / 1024.0)
            nc.vector.tensor_mul(out=sc[0:1, 256:512], in0=mean_r[:],
                                 in1=mean_r[:])
            nc.vector.tensor_sub(out=sc[0:1, 512:768], in0=sc[0:1, 0:256],
                                 in1=sc[0:1, 256:512])
            nc.scalar.activation(out=sc[0:1, 768:1024], in_=sc[0:1, 512:768],
                                 func=AF.Sqrt, bias=eps_t[:])
            rstd_r = stat.tile([1, 256], dtr, tag="rstd_r")
            with nc.allow_low_precision(reason="f32r rounding for matmul"):
                nc.vector.reciprocal(out=rstd_r[:], in_=sc[0:1, 768:1024])
            p_b = ps.tile([128, 1024], dtf, tag="bank")
            nc.tensor.matmul(p_b[:, 0:256], ones_row[:], mean_r[:],
                             start=True, stop=True)
            nc.tensor.matmul(p_b[:, 256:512], ones_row[:], rstd_r[:],
                             start=True, stop=True)
            out_t = act.tile([128, 2048], dtb, tag="ln_out")
            for i in range(8):
                tmp = act2.tile([128, 256], dtr, tag="ln_tmp")
                nc.vector.tensor_sub(out=tmp[:], in0=cs(src_tile[:], i),
                                     in1=p_b[:, 0:256])
                nc.vector.tensor_mul(out=cs(out_t[:], i), in0=tmp[:],
                                     in1=p_b[:, 256:512])
            return out_t

        def ln_param(dram_t, tag, idx=None):
            shp = [dram_t.shape[-2], dram_t.shape[-1]]
            t = lnp.tile(shp, dtf if dram_t.dtype == dtf else dtb, tag=tag)
            nc.gpsimd.dma_start(out=t[:], in_=dram_t[idx] if idx is not None
                                else dram_t[:, :])
            return t

        # persistent vN with ones columns initialized once
        vN = act.tile([128, 16, 130], dtb, tag="vN")
        for tb in range(16):
            nc.vector.tensor_copy(out=vN[:, tb, 64:65], in_=onesb_col[:])
            nc.vector.tensor_copy(out=vN[:, tb, 129:130], in_=onesb_col[:])

        for l in range(L):
            # ---- QKV weights first: stream during LN1 ----
            wqt = wqkv.tile([128, 8, 1024], dtb, tag="wqt")
            nc.sync.dma_start(out=wqt[:],
                              in_=wq_i[l].rearrange("(c p) m -> p c m", p=128))
            wkt = wqkv.tile([128, 8, 1024], dtb, tag="wkt")
            nc.sync.dma_start(out=wkt[:],
                              in_=wk_i[l].rearrange("(c p) m -> p c m", p=128))
            wvt = wqkv.tile([128, 8, 1024], dtb, tag="wvt")
            nc.sync.dma_start(out=wvt[:],
                              in_=wv_i[l].rearrange("(c p) m -> p c m", p=128))
            bqt = ln_param(bq_i, "bqt", l)
            bkt = ln_param(bk_i, "bkt", l)
            vbt = ln_param(vb_i, "vbt", l)

            # ---- LN1 -> h (bf16) ----
            hT = layer_norm(xT)

            # ---- local QKV on own 256 tokens (full weights, bf16) ----
            q_loc = act.tile([128, 8, 256], dtb, tag="q_loc")
            k_loc = act.tile([128, 8, 256], dtb, tag="k_loc")
            v_loc = act.tile([128, 2, 8, 128], dtb, tag="v_loc")
            for name, wt, dst, bt in (("q", wqt, q_loc, bqt),
                                      ("k", wkt, k_loc, bkt)):
                for ht in range(2):
                    p_q = ps.tile([128, 1024], dtf, tag="bank")
                    for c in range(8):
                        for m in range(4):
                            nc.tensor.matmul(
                                p_q[:, 256 * m:256 * (m + 1)],
                                wt[:, c, 128 * (4 * ht + m):128 * (4 * ht + m + 1)],
                                cs(hT[:], c),
                                start=(c == 0 and m % 2 == 0), stop=(c == 7))
                    for m in range(4):
                        j = 4 * ht + m
                        nc.scalar.activation(
                            out=dst[:, j, :],
                            in_=p_q[:, 256 * m:256 * (m + 1)],
                            func=AF.Identity, bias=bt[:, j:j + 1])
            # V token-major: pv[tok, vdim] = sum_feat h[feat, tok] wv[feat, vdim]
            for u in range(2):
                p_v = ps.tile([128, 1024], dtf, tag="bank")
                for c in range(8):
                    for s2 in range(2):
                        nc.tensor.matmul(
                            p_v[:, 512 * s2:512 * (s2 + 1)],
                            hT[:, 256 * c + 128 * u:256 * c + 128 * (u + 1)],
                            wvt[:, c, 512 * s2:512 * (s2 + 1)],
                            start=(c == 0), stop=False)
                for s2 in range(2):
                    # + be1 contribution: ones(tok) x (Wv^T be1)[vdim]
                    nc.tensor.matmul(
                        p_v[:, 512 * s2:512 * (s2 + 1)],
                        onesb_row[0:1, 0:128],
                        vbt[0:1, 512 * s2:512 * (s2 + 1)],
                        start=False, stop=True)
                nc.scalar.copy(
                    out=v_loc[:, u].rearrange("p j d -> p (j d)"), in_=p_v[:])

            # ---- qkv AllToAll: block j (384 rows) = q|k (feature-major,
            #      dims 128j..) + v (token-major, cols = half0|half1 dims) ----
            a2a_in = dram.tile([NCORES * 384, TPC], dtb, tag="a2a_qkv_in")
            stg = a2a_in[:, :].rearrange("(j p) t -> p j t", p=384)
            nc.gpsimd.dma_start(out=stg[0:128], in_=q_loc[:])
            nc.gpsimd.dma_start(out=stg[128:256], in_=k_loc[:])
            stg_v = stg[256:384].rearrange("p j (u d) -> p u j d", u=2)
            for u in range(2):
                nc.gpsimd.dma_start(out=stg_v[:, u], in_=v_loc[:, u])
            a2a_out = dram.tile([NCORES * 384, TPC], dtb, tag="a2a_qkv_out")
            nc.gpsimd.collective_compute(
                "AllToAll", mybir.AluOpType.bypass, replica_groups=RG,
                ins=[a2a_in.opt()], outs=[a2a_out.opt()])

            # ---- Wo weights: stream during A2A + attention ----
            bot = ln_param(bo_i, "bot", l)
            wot = wff.tile([128, 8, 1024], dtb, tag="w1gt")
            nc.sync.dma_start(
                out=wot[:], in_=wo_i[l].rearrange("(c p) n -> p c n", p=128))

            # receive: my 2 heads (128 dims), all 2048 tokens
            qT = act.tile([128, 2048], dtb, tag="qT")
            kT = act.tile([128, 2048], dtb, tag="kT")
            rcv = a2a_out[:, :].rearrange("(j p) t -> p j t", p=384)
            nc.gpsimd.dma_start(
                out=qT[:].rearrange("p (j t) -> p j t", j=8), in_=rcv[0:128])
            nc.gpsimd.dma_start(
                out=kT[:].rearrange("p (j t) -> p j t", j=8), in_=rcv[128:256])
            rcv_v = rcv[256:384].rearrange("p j (u d) -> p u j d", u=2)
            for u in range(2):
                nc.gpsimd.dma_start(
                    out=vN[:, :, 0:64].rearrange(
                        "p (j u) d -> p u j d", u=2)[:, u],
                    in_=rcv_v[:, u, :, 0:64])
                nc.gpsimd.dma_start(
                    out=vN[:, :, 65:129].rearrange(
                        "p (j u) d -> p u j d", u=2)[:, u],
                    in_=rcv_v[:, u, :, 64:128])

            # ---- attention per (head, batch); scores transposed [k, q] ----
            ctx_sb = act.tile([128, 2048], dtb, tag="ctx_sb")
            for hh in range(2):
                for b in range(B):
                    qs = qT[64 * hh:64 * (hh + 1), 1024 * b:1024 * (b + 1)]
                    ks = kT[64 * hh:64 * (hh + 1), 1024 * b:1024 * (b + 1)]
                    p_u = ps.tile([128, 1024], dtf, tag="bank")
                    for kb in range(8):
                        live = 1024 - 128 * kb
                        p_sc = ps.tile([128, 1024], dtf, tag="bank")
                        off = 0
                        while off < live:
                            w = min(512, live - off)
                            nc.tensor.matmul(
                                p_sc[:, off:off + w],
                                ks[:, 128 * kb:128 * (kb + 1)],
                                qs[:, 128 * kb + off:128 * kb + off + w],
                                start=True, stop=True)
                            off += w
                        nc.vector.tensor_add(out=p_sc[:, 0:128],
                                             in0=p_sc[:, 0:128], in1=causal[:])
                        es = expp.tile([128, 1024], dtb, tag="expS")
                        nc.scalar.activation(
                            out=es[:, 0:live], in_=p_sc[:, 0:live],
                            func=AF.Exp,
                            bias=padb[:, 8 * b + kb:8 * b + kb + 1])
                        # U = [1 | V].T @ expS accumulated over k blocks
                        vsl = vN[:, 8 * b + kb, 65 * hh:65 * (hh + 1)]
                        off = 0
                        while off < live:
                            w = min(512, live - off)
                            nc.tensor.matmul(
                                p_u[0:65, 128 * kb + off:128 * kb + off + w],
                                vsl, es[:, off:off + w],
                                start=(kb == 0), stop=(kb == 7))
                            off += w
                    # rows: 0:64 = unnormalized ctx, 64 = sum(exp)
                    rc = stat.tile([1, 1024], dtb, tag="rc")
                    with nc.allow_low_precision(reason="softmax norm bf16"):
                        nc.vector.reciprocal(out=rc[:], in_=p_u[64:65, :])
                    rbb = stat.tile([64, 1024], dtb, tag="rbb")
                    nc.gpsimd.partition_broadcast(rbb[:], rc[:])
                    nc.vector.tensor_mul(
                        out=ctx_sb[64 * hh:64 * (hh + 1),
                                   1024 * b:1024 * (b + 1)],
                        in0=p_u[0:64, :], in1=rbb[:])

            # ---- ctx AllToAll: shard j = my heads x rank-j tokens ----
            a2a2_in = dram.tile([NCORES * 128, TPC], dtb, tag="a2a_ctx_in")
            nc.gpsimd.dma_start(
                out=a2a2_in[:, :].rearrange("(j p) t -> p j t", p=128),
                in_=ctx_sb[:].rearrange("p (j t) -> p j t", j=8))
            a2a2_out = dram.tile([NCORES * 128, TPC], dtb, tag="a2a_ctx_out")
            nc.gpsimd.collective_compute(
                "AllToAll", mybir.AluOpType.bypass, replica_groups=RG,
                ins=[a2a2_in.opt()], outs=[a2a2_out.opt()])
            ctxf = act.tile([128, 8, 256], dtb, tag="ctxf")
            nc.gpsimd.dma_start(
                out=ctxf[:],
                in_=a2a2_out[:, :].rearrange("(c p) t -> p c t", p=128))

            # ---- Wo + bias + residual (own tokens) ----
            for m in range(8):
                p_y = ps.tile([128, 1024], dtf, tag="bank")
                for c in range(8):
                    nc.tensor.matmul(p_y[:, 0:256],
                                     wot[:, c, 128 * m:128 * (m + 1)],
                                     ctxf[:, c, :],
                                     start=(c == 0), stop=(c == 7))
                tmp = act2.tile([128, 256], dtr, tag="res_tmp")
                nc.scalar.activation(out=tmp[:], in_=p_y[:, 0:256],
                                     func=AF.Identity, bias=bot[:, m:m + 1])
                nc.vector.tensor_add(out=cs(xT[:], m), in0=cs(xT[:], m),
                                     in1=tmp[:])

            # ---- LN2 + FFN ----
            h2T = layer_norm(xT)
            b1t = ln_param(b1_i, "b1t", l)
            b2t = ln_param(b2_i, "b2t", l)
            # y2 accumulates in PSUM across all 4 F-groups
            p_y2a = ps.tile([128, 1024], dtf, tag="bank")
            p_y2b = ps.tile([128, 1024], dtf, tag="bank")
            for g in range(4):
                w1gt = wff.tile([128, 8, 1024], dtb, tag="w1gt")
                nc.scalar.dma_start(
                    out=w1gt[:],
                    in_=w1_i[l][:, 1024 * g:1024 * (g + 1)].rearrange(
                        "(c p) f -> p c f", p=128))
                w2gt = wff.tile([128, 8, 1024], dtb, tag="w2gt")
                nc.scalar.dma_start(
                    out=w2gt[:],
                    in_=w2_i[l][1024 * g:1024 * (g + 1), :].rearrange(
                        "(f p) m -> p f m", p=128))
                p_u1 = ps.tile([128, 1024], dtf, tag="bank")
                p_u2 = ps.tile([128, 1024], dtf, tag="bank")
                for c in range(8):
                    for fl in range(8):
                        pu = p_u1 if fl < 4 else p_u2
                        nc.tensor.matmul(
                            pu[:, 256 * (fl % 4):256 * (fl % 4 + 1)],
                            w1gt[:, c, 128 * fl:128 * (fl + 1)], cs(h2T[:], c),
                            start=(c == 0 and fl % 2 == 0), stop=(c == 7))
                guT = act2.tile([128, 2048], dtb, tag="guT")
                for fl in range(8):
                    pu = p_u1 if fl < 4 else p_u2
                    fc = 8 * g + fl
                    nc.scalar.activation(
                        out=cs(guT[:], fl),
                        in_=pu[:, 256 * (fl % 4):256 * (fl % 4 + 1)],
                        func=AF.Gelu, bias=b1t[:, fc:fc + 1])
                for fl in range(8):
                    for mq in range(8):
                        py = p_y2a if mq < 4 else p_y2b
                        nc.tensor.matmul(
                            py[:, 256 * (mq % 4):256 * (mq % 4 + 1)],
                            w2gt[:, fl, 128 * mq:128 * (mq + 1)], cs(guT[:], fl),
                            start=(g == 0 and fl == 0 and mq % 2 == 0),
                            stop=(g == 3 and fl == 7))
            for m in range(8):
                py = p_y2a if m < 4 else p_y2b
                tmp2 = act2.tile([128, 256], dtr, tag="res_tmp")
                nc.scalar.activation(
                    out=tmp2[:], in_=py[:, 256 * (m % 4):256 * (m % 4 + 1)],
                    func=AF.Identity, bias=b2t[:, m:m + 1])
                nc.vector.tensor_add(out=cs(xT[:], m), in0=cs(xT[:], m),
                                     in1=tmp2[:])

        # ---- final LN + token-sharded LM head (no collective) ----
        xfT = layer_norm(xT)
        boutt = ln_param(bout_i, "boutt")
        for vg in range(VPAD // 1024):
            wvh = wqkv.tile([128, 8, 1024], dtb,
                            tag=("wqt", "wkt", "wvt")[vg % 3])
            nc.sync.dma_start(
                out=wvh[:],
                in_=wout_i.ap()[:, 1024 * vg:1024 * (vg + 1)].rearrange(
                    "(c p) m -> p c m", p=128))
            osb = act2.tile([128, 2048], dtb, tag="osb")
            for q8 in range(8):
                vm = 8 * vg + q8
                p_o = ps.tile([128, 1024], dtf, tag="bank")
                for c in range(8):
                    nc.tensor.matmul(
                        p_o[:, 0:256], wvh[:, c, 128 * q8:128 * (q8 + 1)],
                        cs(xfT[:], c),
                        start=(c == 0), stop=(c == 7))
                nc.scalar.activation(
                    out=osb[:, 256 * q8:256 * (q8 + 1)], in_=p_o[:, 0:256],
                    func=AF.Identity, bias=boutt[:, vm:vm + 1])
            nc.sync.dma_start(
                out=out_o[:, 2048 * vg:2048 * (vg + 1)], in_=osb[:])


def _host_inputs(inputs):
    bf16 = ml_dtypes.bfloat16
    tokens = np.asarray(inputs["tokens"])
    emb = np.asarray(inputs["emb"], dtype=np.float32)
    pe = np.asarray(inputs["pe"], dtype=np.float32)
    Wq = np.asarray(inputs["Wq"], dtype=np.float32)
    Wk = np.asarray(inputs["Wk"], dtype=np.float32)
    Wv = np.asarray(inputs["Wv"], dtype=np.float32)
    Wo = np.asarray(inputs["Wo"], dtype=np.float32)
    bo = np.asarray(inputs["bo"], dtype=np.float32)
    g1 = np.asarray(inputs["g1"], dtype=np.float32)
    be1 = np.asarray(inputs["be1"], dtype=np.float32)
    g2 = np.asarray(inputs["g2"], dtype=np.float32)
    be2 = np.asarray(inputs["be2"], dtype=np.float32)
    W1 = np.asarray(inputs["W1"], dtype=np.float32)
    b1 = np.asarray(inputs["b1"], dtype=np.float32)
    W2 = np.asarray(inputs["W2"], dtype=np.float32)
    b2 = np.asarray(inputs["b2"], dtype=np.float32)
    gf = np.asarray(inputs["gf"], dtype=np.float32)
    bf = np.asarray(inputs["bf"], dtype=np.float32)
    Wout = np.asarray(inputs["Wout"], dtype=np.float32)
    bout = np.asarray(inputs["bout"], dtype=np.float32)

    x0 = emb[tokens] * math.sqrt(float(D)) + pe[:S][None]   # (B, S, D)
    xflat = np.ascontiguousarray(x0.reshape(T, D))

    padb = np.where(tokens == 0, np.float32(NEG), np.float32(0.0))
    padb = np.ascontiguousarray(padb.reshape(2, 8, 128).astype(np.float32))
    r = np.arange(128)
    causal = np.where(r[:, None] > r[None, :], np.float32(NEG),
                      np.float32(0.0)).astype(np.float32)

    def tchunks1(a, n):   # [D'] -> [128, n] feature-major chunks
        return np.ascontiguousarray(a.reshape(n, 128).T.astype(np.float32))

    def tchunks(a, n):   # [L, D'] -> [L, 128, n] feature-major chunks
        return np.ascontiguousarray(
            a.reshape(L, n, 128).transpose(0, 2, 1).astype(np.float32))

    # fold LN gammas into consumer weights; betas into bias vectors
    wq_s = (Wq * g1[:, :, None] / math.sqrt(float(DK))).astype(np.float32)
    wk_s = (Wk * g1[:, :, None]).astype(np.float32)
    wv_s = (Wv * g1[:, :, None]).astype(np.float32)
    w1_s = (W1 * g2[:, :, None]).astype(np.float32)
    wout_s = (Wout * gf[:, None]).astype(np.float32)

    bq = np.einsum('lde,ld->le', Wq, be1) / math.sqrt(float(DK))
    bk = np.einsum('lde,ld->le', Wk, be1)
    bv = np.einsum('lde,ld->le', Wv, be1)
    b1_eff = b1 + np.einsum('ldf,ld->lf', W1, be2)
    bout_eff = bout + np.einsum('dv,d->v', Wout, bf)

    bqt = tchunks(bq, 8)
    bkt = tchunks(bk, 8)
    bot, b2t = tchunks(bo, 8), tchunks(b2, 8)
    b1t = tchunks(b1_eff, 32)

    wout_p = np.zeros((D, VPAD), dtype=np.float32)
    wout_p[:, :V] = wout_s
    bout_p = np.zeros((VPAD,), dtype=np.float32)
    bout_p[:V] = bout_eff

    shared = dict(
        wq=np.ascontiguousarray(wq_s.astype(bf16)),
        wk=np.ascontiguousarray(wk_s.astype(bf16)),
        wv=np.ascontiguousarray(wv_s.astype(bf16)),
        wo=np.ascontiguousarray(Wo.astype(bf16)),
        w1=np.ascontiguousarray(w1_s.astype(bf16)),
        w2=np.ascontiguousarray(W2.astype(bf16)),
        wout=np.ascontiguousarray(wout_p.astype(bf16)),
        bout=np.ascontiguousarray(
            bout_p.reshape(256, 128).T.astype(np.float32)),
        bq=bqt, bk=bkt,
        vb=np.ascontiguousarray(bv.reshape(L, 1, 1024).astype(bf16)),
        bo=bot, b1=b1t, b2=b2t,
        causal=causal, padb=padb,
    )
    in_maps = []
    for c in range(NCORES):
        m = dict(shared)
        m["x0t"] = np.ascontiguousarray(
            xflat[TPC * c:TPC * (c + 1)].T)
        in_maps.append(m)
    return in_maps


def kernel(**inputs):
    from concourse import bass_utils

    if "nc" not in _CACHE:
        _CACHE["nc"] = _build()
    nc = _CACHE["nc"]
    in_maps = _host_inputs(inputs)
    res = bass_utils.run_bass_kernel_spmd(
        nc, in_maps, core_ids=list(range(NCORES)))
    full = np.empty((T, V), dtype=np.float32)
    for c in range(NCORES):
        lt = np.asarray(res.results[c]["logits_t"])  # [128, 65536] bf16
        # vocab v = 1024*vg + 128*q8 + p at [p, 2048*vg + 256*q8 + t]
        lv = lt.reshape(128, 32, 8, TPC).transpose(1, 2, 0, 3).reshape(
            VPAD, TPC)
        full[TPC * c:TPC * (c + 1), :] = lv[:V, :].T.astype(np.float32)
    return full.reshape(B, S, V)


if __name__ == "__main__":
    import sys
    sys.path.insert(0, "/root/problem")
    import reference
    inp = reference.setup_inputs()
    out = kernel(**{k: np.asarray(v) for k, v in inp.items()})
    print("kernel output", out.shape, out.dtype)
